# revision 1
# baseline (speedup 1.0000x reference)
"""Trainium2 Bass kernel for a contextual loss (cosine-distance softmin loss).

Math (per batch b):
  mu_c      = mean_n Y[b,c,n]
  xc = X-mu, yc = Y-mu                      (centered, [C,N])
  G[i,j]    = <xc_i, yc_j>                  (K=C=64 matmul)
  s[i,j]    = rx_i * ry_j * G[i,j]          (cosine similarity; rx/ry = 1/norms)
  dmin_i    = 1 - max_j s[i,j]
  a_i       = 1 / (H * (dmin_i + EPS_MIN))
  CX_i      = max_j A / sum_j A = 1 / sum_j exp(a_i*(s_ij - smax_i))
  loss_b    = -log(mean_i CX_i)

Sharding: 8 cores = 4 batches x 2 row-halves. Each core gets its full-batch
Y [64,4096] and its half of X's columns [64,2048], computes
S'_i = sum_j exp(...) for its 2048 rows, returns S' as [128,16]
(partition p, chunk k  <->  row k*128+p). Host reduces to the [4] loss.

On-device pipeline per 128-row chunk:
  PE   : 8 matmuls (f32r, K=64, N=512) -> PSUM [128,2048] x2
  DVE  : tensor_tensor_reduce fuses (G * ry_bcast) -> SBUF rowbuf copy
         with a running row-max (accum), then tiny per-row scalar chain
  ACT  : exp(scale*q + bias) with per-partition scale/bias and
         accumulated row-sum (accum_out) -> S' contributions
"""

import math

import numpy as np

import concourse.bacc as bacc
import concourse.mybir as mybir
from concourse.dve_ops import TENSOR_MASK_REDUCE
from concourse.bass_utils import run_bass_kernel_spmd
from concourse.mybir import ActivationFunctionType as AF, AluOpType as OP, AxisListType
from concourse.tile import TileContext

F32 = mybir.dt.float32
F32R = mybir.dt.float32r

B, C, N = 4, 64, 4096          # batch, channels, spatial (64*64)
NX = N // 2                    # rows per core (half batch)
CH = NX // 128                 # 16 chunks of 128 rows
HALF = N // 2                  # column half processed per DVE op
H_BAND = 5.0
EPS_MIN = 1e-3
LN02 = math.log(0.2)           # fold the 1/H into rx via exp(... + ln(1/H))

_NC_CACHE = {}


def build_nc():
    nc = bacc.Bacc("TRN2", target_bir_lowering=False, debug=False, num_devices=8)
    x_d = nc.dram_tensor("Xh", [C, NX], F32, kind="ExternalInput")
    y_d = nc.dram_tensor("Yb", [C, N], F32, kind="ExternalInput")
    out_d = nc.dram_tensor("out", [128, CH], F32, kind="ExternalOutput")

    with TileContext(nc) as tc:
        with (
            tc.tile_pool(name="persist", bufs=1) as persist,
            tc.tile_pool(name="mm", bufs=2, space="PSUM") as mmpool,
            tc.tile_pool(name="rb", bufs=4) as rbpool,
            tc.tile_pool(name="small", bufs=4) as small,
            tc.tile_pool(name="escr", bufs=2) as escrpool,
        ):
            # ---------------- load inputs ----------------
            y_sb = persist.tile([C, N], F32)
            nc.sync.dma_start(out=y_sb[:], in_=y_d[:])
            x_sb = persist.tile([C, NX], F32)
            nc.sync.dma_start(out=x_sb[:], in_=x_d[:])

            ones_f = persist.tile([C, 128], F32)
            nc.vector.memset(ones_f[:], 1.0)
            ones_w = persist.tile([C, 128], F32R)
            nc.vector.tensor_copy(ones_w[:], ones_f[:])
            ones1 = persist.tile([C, 2], F32R)
            nc.vector.tensor_copy(ones1[:], ones_f[:, 0:2])

            # ---------------- center by Y's spatial mean ----------------
            musum = small.tile([C, 1], F32, tag="musum")
            nc.vector.reduce_sum(out=musum[:], in_=y_sb[:], axis=AxisListType.X)
            mu = small.tile([C, 1], F32, tag="mu")
            nc.vector.tensor_scalar_mul(mu[:], musum[:], 1.0 / N)

            ycen = persist.tile([C, N], F32)
            nc.vector.tensor_scalar(ycen[:], y_sb[:], mu[:], None, OP.subtract)
            xcen = persist.tile([C, NX], F32R)
            nc.vector.tensor_scalar(xcen[:], x_sb[:], mu[:], None, OP.subtract)

            ysq = persist.tile([C, N], F32R)
            nc.scalar.activation(ysq[:], ycen[:], AF.Square)
            xsq = persist.tile([C, NX], F32R)
            nc.scalar.activation(xsq[:], xcen[:], AF.Square)

            # ---------------- ry broadcast [128, N] ----------------
            # ones[64,128].T @ ysq[64,512] = column sums of ysq, replicated
            # down all 128 partitions.  ry = 1/sqrt(ny2) done as exp(-.5*ln).
            ry_bc = persist.tile([128, N], F32)
            for h in range(2):
                ps = mmpool.tile([128, HALF], F32, tag="mm")
                for j in range(4):
                    c0 = h * HALF + j * 512
                    nc.tensor.matmul(
                        ps[:, j * 512:(j + 1) * 512],
                        lhsT=ones_w[:],
                        rhs=ysq[:, c0:c0 + 512],
                        start=True, stop=True,
                    )
                tln = escrpool.tile([128, HALF], F32, tag="escr")
                nc.scalar.activation(tln[:], ps[:], AF.Ln)
                nc.scalar.activation(
                    ry_bc[:, h * HALF:(h + 1) * HALF], tln[:], AF.Exp, scale=-0.5
                )

            # ---------------- rx5 = 0.2 * 1/sqrt(nx2)  [128, CH] ----------------
            # xsq[64,128chunk].T @ ones[64,1] = per-row ||xc_i||^2 in
            # [128 rows, chunk] layout.
            nx2 = mmpool.tile([128, 2 * CH], F32, tag="mm")
            for k in range(CH):
                nc.tensor.matmul(
                    nx2[:, 2 * k:2 * k + 2],
                    lhsT=xsq[:, k * 128:(k + 1) * 128],
                    rhs=ones1[:],
                    start=True, stop=True,
                )
            yhat = persist.tile([C, N], F32R)
            nc.vector.tensor_tensor(yhat[:], ycen[:], ry_bc[:C, :], OP.mult)
            c3big = persist.tile([128, 1], F32)
            nc.vector.memset(c3big[:], 1.0e9)

            tn = small.tile([128, CH], F32, tag="tn")
            nc.scalar.activation(
                tn[:], nx2[:].rearrange("p (k two) -> p k two", two=2)[:, :, 0], AF.Ln
            )
            ln02 = persist.tile([128, 1], F32)
            nc.vector.memset(ln02[:], LN02)
            rx5 = persist.tile([128, CH], F32)
            nc.scalar.activation(rx5[:], tn[:], AF.Exp, bias=ln02[:], scale=-0.5)

            # ---------------- main loop ----------------
            ssums = persist.tile([128, 2 * CH], F32)
            for k in range(CH):
                lhs = xcen[:, k * 128:(k + 1) * 128]
                pm = small.tile([128, 2], F32, tag="pm")
                rbs = []
                for h in range(2):
                    ps = mmpool.tile([128, HALF], F32, tag="mm")
                    for j in range(4):
                        c0 = h * HALF + j * 512
                        nc.tensor.matmul(
                            ps[:, j * 512:(j + 1) * 512],
                            lhsT=lhs,
                            rhs=yhat[:, c0:c0 + 512],
                            start=True, stop=True,
                        )
                    rb = rbpool.tile([128, HALF], F32, tag="rb")
                    init = -3.0e38 if h == 0 else pm[:, 0:1]
                    # rb = copy(ps); pm[:,h] = max(row-max(rb), init)
                    # (custom-DVE mask-reduce with an all-pass window)
                    nc.vector._custom_dve(
                        TENSOR_MASK_REDUCE,
                        out=rb[:],
                        in0=ps[:],
                        in1=c3big[:],
                        s0=0.0,
                        s1=init,
                        imm2=1.0,
                        accum_out=pm[:, h:h + 1],
                    )
                    rbs.append(rb)

                # per-row constants: a' = rx5 / (1.001 - 5*rx5*pmax), bias = -a'*pmax
                smax = small.tile([128, 1], F32, tag="smax")
                nc.vector.tensor_scalar(
                    smax[:], pm[:, 1:2], rx5[:, k:k + 1], H_BAND, OP.mult, OP.mult
                )
                den = small.tile([128, 1], F32, tag="den")
                nc.vector.tensor_scalar(
                    den[:], smax[:], -1.0, 1.0 + EPS_MIN, OP.mult, OP.add
                )
                rec = small.tile([128, 1], F32, tag="rec")
                nc.vector.reciprocal(rec[:], den[:])
                aa = small.tile([128, 1], F32, tag="aa")
                nc.vector.tensor_scalar(aa[:], rec[:], rx5[:, k:k + 1], None, OP.mult)
                bb = small.tile([128, 1], F32, tag="bb")
                nc.vector.tensor_scalar(
                    bb[:], aa[:], pm[:, 1:2], -1.0, OP.mult, OP.mult
                )

                for h in range(2):
                    es = escrpool.tile([128, HALF], F32, tag="escr")
                    nc.scalar.activation(
                        es[:],
                        rbs[h][:],
                        AF.Exp,
                        bias=bb[:],
                        scale=aa[:],
                        accum_out=ssums[:, 2 * k + h:2 * k + h + 1],
                    )

            # ---------------- finalize ----------------
            sfin = persist.tile([128, CH], F32)
            nc.vector.reduce_sum(
                out=sfin[:],
                in_=ssums[:].rearrange("p (k t) -> p k t", t=2),
                axis=AxisListType.X,
            )
            nc.sync.dma_start(out=out_d[:], in_=sfin[:])

    nc.compile()
    return nc


def _get_nc():
    if "nc" not in _NC_CACHE:
        _NC_CACHE["nc"] = build_nc()
    return _NC_CACHE["nc"]


def make_in_maps(X_features, Y_features):
    X = np.ascontiguousarray(np.asarray(X_features, np.float32).reshape(B, C, N))
    Y = np.ascontiguousarray(np.asarray(Y_features, np.float32).reshape(B, C, N))
    in_maps = []
    for c in range(8):
        b, h = divmod(c, 2)
        in_maps.append({
            "Xh": np.ascontiguousarray(X[b, :, h * NX:(h + 1) * NX]),
            "Yb": Y[b],
        })
    return in_maps


def combine(results):
    """results: list of 8 dicts with 'out' [128, CH] = S' per row."""
    out = np.empty(B, np.float32)
    for b in range(B):
        tot = 0.0
        for h in range(2):
            s = results[2 * b + h]["out"].astype(np.float64)
            tot += (1.0 / s).sum()
        out[b] = -np.log(tot / N)
    return out


def kernel(X_features, Y_features):
    nc = _get_nc()
    in_maps = make_in_maps(X_features, Y_features)
    res = run_bass_kernel_spmd(nc, in_maps, core_ids=list(range(8)))
    return combine(res.results)


if __name__ == "__main__":
    rng = np.random.default_rng(0)
    X = rng.standard_normal((B, C, 64, 64)).astype(np.float32)
    Y = rng.standard_normal((B, C, 64, 64)).astype(np.float32)
    print(kernel(X_features=X, Y_features=Y))

